# revision 42
# baseline (speedup 1.0000x reference)
"""Trainium2 Bass kernel for nn_HGNER (windowed bi-LSTM + attention + linear head).

Sharding: 8 cores x 128 tokens (data-parallel over the flattened (B,L) token
axis; each core gets half of one batch row plus a 4-token halo). Small LSTM /
linear params are replicated to every core.

Layout inside a core: "feature-partition" — SBUF partitions carry a 128-wide
feature chunk, the free dim packs (chunk, token). This makes the recurrent
h @ W_hh matmuls transpose-free and keeps elementwise ops on long free dims.

Perf structure vs the straightforward version:
 - All heavy matmuls run in fp8-e4m3 with DoubleRow perf mode (2 contraction
   rows per PE pass). Weights are pre-scaled by 16 on the host (to dodge fp8
   subnormals); the 1/16 descale rides the ACT engine's free input scale.
 - Gate order is host-permuted to [i,f,o,g] so each step needs one sigmoid
   pass (3H wide) + one tanh pass instead of three activation calls.
 - The per-step state masking (ragged window edges) is replaced by: (a) G
   columns at invalid halo positions forced to -224 (=> sigma,tanh saturate,
   state becomes 0 which matches "keep zero state" on the leading side), and
   (b) for the trailing side, per-step single-token-column snapshots of h
   plus a tiny 4-column predicated merge at chain end.
 - G's PSUM->SBUF eviction is a DVE tensor_scalar that also folds in the
   LSTM bias as a per-partition scalar (no bias matmuls).
 - Windows run largest-first so the small windows' G matmuls and weight DMA
   fill the tensor-engine gaps of the latency-bound big-window steps; the
   first window's weights are DMA'd ahead of all non-critical constants.
"""

import numpy as np
import ml_dtypes

import concourse.bass as bass
import concourse.bacc as bacc_mod
import concourse.mybir as mybir
from concourse.tile import TileContext
from concourse.bass_utils import run_bass_kernel_spmd

F32 = mybir.dt.float32
BF16 = mybir.dt.bfloat16
FP8 = mybir.dt.float8e4
U8 = mybir.dt.uint8
AF = mybir.ActivationFunctionType
ALU = mybir.AluOpType
AX = mybir.AxisListType
DR = mybir.MatmulPerfMode.DoubleRow

B, L, D, H, NW, NL = 4, 256, 768, 384, 4, 9
WINDOWS = (3, 5, 7, 9)
NCORES = 8
TPC = 128          # tokens per core
HALO = 4           # max half-window
TH = TPC + 2 * HALO  # 136 tokens incl. halo
DC = D // 128      # 6 chunks of input features
HC = H // 128      # 3 chunks of hidden features
GC = 4 * H // 128  # 12 chunks of gate features
NCH = 2 * NW       # 8 (window, direction) chains
GLEN = GC * TH     # G region length in the per-chain state tile
HOFF = GLEN        # h region offset (h lives beside G so one DoubleRow
                   # matmul can contract (id8, whh_k2) x (G_slice, h_k2))
GTOT = GLEN + HC * TPC
SCALE = 1.0 / np.sqrt(np.float32(D))
WS = 16.0          # host-side weight scale (fp8 subnormal avoidance)
IWS = 1.0 / WS
EDGE = -224.0      # fp8-storable; /16 => sigma(-14)~8e-7, tanh(-14)~-1

_CACHE = {}


def _build():
    nc = bacc_mod.Bacc()

    # ---- DRAM I/O ----
    xt_d = nc.dram_tensor("xt", [D, TH], FP8, kind="ExternalInput")
    xb_d = nc.dram_tensor("xb", [D, TPC], BF16, kind="ExternalInput")
    wih_d = nc.dram_tensor("wih", [NCH, D, 4 * H], FP8, kind="ExternalInput")
    whh_d = nc.dram_tensor("whh", [NCH, H, 4 * H], FP8, kind="ExternalInput")
    # per chain: [id8 | whh_k2] plane-pairs per gate chunk, for the fused
    # (G-inject + 3rd h-chunk) DoubleRow matmul
    whh2_d = nc.dram_tensor("whh2", [NCH, 128, GC * 2 * 128], FP8,
                            kind="ExternalInput")
    bias_d = nc.dram_tensor("bias", [128, NCH * GC], F32, kind="ExternalInput")
    linw_d = nc.dram_tensor("linw", [D, NL], BF16, kind="ExternalInput")
    linb_d = nc.dram_tensor("linb", [1, NL], BF16, kind="ExternalInput")
    id8_d = nc.dram_tensor("id8", [128, 128], FP8, kind="ExternalInput")
    idb_d = nc.dram_tensor("idb", [128, 128], BF16, kind="ExternalInput")
    ones_d = nc.dram_tensor("ones", [128, 1], BF16, kind="ExternalInput")
    onesr_d = nc.dram_tensor("onesr", [1, 512], BF16, kind="ExternalInput")
    # edge predication masks (per-core data; program is identical on all cores)
    pgl_d = nc.dram_tensor("pgl", [128, GC * 4], U8, kind="ExternalInput")
    pgr_d = nc.dram_tensor("pgr", [128, GC * 4], U8, kind="ExternalInput")
    pmf_d = nc.dram_tensor("pmf", [128, NW * HC * 4], U8, kind="ExternalInput")
    pmb_d = nc.dram_tensor("pmb", [128, NW * HC * 4], U8, kind="ExternalInput")
    out_d = nc.dram_tensor("out", [NL, TPC], F32, kind="ExternalOutput")

    with TileContext(nc) as tc:
        with (
            tc.tile_pool(name="const", bufs=1) as cpool,
            tc.tile_pool(name="wih", bufs=2) as wih_pool,
            tc.tile_pool(name="whh", bufs=6) as whh_pool,
            tc.tile_pool(name="g", bufs=8) as g_pool,
            tc.tile_pool(name="muti", bufs=NCH + 2) as muti_pool,
            tc.tile_pool(name="st", bufs=6) as st_pool,
            tc.tile_pool(name="snap", bufs=4) as snap_pool,
            tc.tile_pool(name="tmp", bufs=8) as tmp_pool,
            tc.tile_pool(name="fin", bufs=2) as fin_pool,
            tc.tile_pool(name="ps", bufs=1, space="PSUM") as ps_pool,
            tc.tile_pool(name="psg", bufs=2, space="PSUM") as psg_pool,
        ):
            def load_chain_weights(c):
                wihT = wih_pool.tile([128, DC * 4 * H], FP8, tag="wih")
                nc.sync.dma_start(
                    out=wihT[:].rearrange("p (k n) -> p k n", k=DC),
                    in_=wih_d[c].rearrange("(k p) n -> p k n", p=128),
                )
                whhT = whh_pool.tile([128, 2 * 4 * H], FP8, tag="whh")
                nc.sync.dma_start(
                    out=whhT[:].rearrange("p (k n) -> p k n", k=2),
                    in_=whh_d[c, 0:2 * 128].rearrange(
                        "(k p) n -> p k n", p=128),
                )
                whh2T = whh_pool.tile([128, GC * 2 * 128], FP8, tag="whh2")
                nc.sync.dma_start(out=whh2T[:], in_=whh2_d[c])
                return (wihT, whhT, whh2T)

            # ---- critical-path DMAs first: x, first window's weights ----
            xt = cpool.tile([128, DC * TH], FP8, tag="xt")
            nc.sync.dma_start(
                out=xt[:].rearrange("p (k t) -> p k t", t=TH),
                in_=xt_d[:].rearrange("(k p) t -> p k t", p=128),
            )
            biasr = cpool.tile([128, NCH * GC], F32, tag="bias")
            nc.sync.dma_start(out=biasr[:], in_=bias_d[:])
            prefetched = {6: load_chain_weights(6), 7: load_chain_weights(7)}
            pgl = cpool.tile([128, GC * 4], U8, tag="pgl")
            nc.sync.dma_start(out=pgl[:], in_=pgl_d[:])
            pgr = cpool.tile([128, GC * 4], U8, tag="pgr")
            nc.sync.dma_start(out=pgr[:], in_=pgr_d[:])
            id8 = cpool.tile([128, 128], FP8, tag="id8")
            nc.sync.dma_start(out=id8[:], in_=id8_d[:])
            # ---- non-critical consts (epilogue / later windows) ----
            xb = cpool.tile([128, DC * TPC], BF16, tag="xb")
            nc.sync.dma_start(
                out=xb[:].rearrange("p (k t) -> p k t", t=TPC),
                in_=xb_d[:].rearrange("(k p) t -> p k t", p=128),
            )
            linw = cpool.tile([128, DC * NL], BF16, tag="linw")
            nc.sync.dma_start(
                out=linw[:].rearrange("p (k n) -> p k n", n=NL),
                in_=linw_d[:].rearrange("(k p) n -> p k n", p=128),
            )
            linb = cpool.tile([1, NL], BF16, tag="linb")
            nc.sync.dma_start(out=linb[:], in_=linb_d[:])
            idb = cpool.tile([128, 128], BF16, tag="idb")
            nc.sync.dma_start(out=idb[:], in_=idb_d[:])
            ones = cpool.tile([128, 1], BF16, tag="ones")
            nc.sync.dma_start(out=ones[:], in_=ones_d[:])
            onesr = cpool.tile([1, 512], BF16, tag="onesr")
            nc.sync.dma_start(out=onesr[:], in_=onesr_d[:])
            pmf = cpool.tile([128, NW * HC * 4], U8, tag="pmf")
            nc.sync.dma_start(out=pmf[:], in_=pmf_d[:])
            pmb = cpool.tile([128, NW * HC * 4], U8, tag="pmb")
            nc.sync.dma_start(out=pmb[:], in_=pmb_d[:])
            negt = cpool.tile([128, GC * 4], FP8, tag="negt")
            nc.vector.memset(negt[:], EDGE)

            # one-time engine touches of DMA-loaded consts: collapse later
            # waits to a single semaphore (instr structs have 1 wait slot)
            wu8 = cpool.tile([128, 1], U8, tag="wu8")
            nc.vector.tensor_copy(wu8[:], pgl[:, 0:1])
            nc.vector.tensor_copy(wu8[:], pgr[:, 0:1])
            nc.vector.tensor_copy(wu8[:], pmf[:, 0:1])
            nc.vector.tensor_copy(wu8[:], pmb[:, 0:1])
            wb0 = cpool.tile([128, 1], BF16, tag="wb0")
            nc.vector.tensor_copy(wb0[:], xb[:, 0:1])
            wb1 = cpool.tile([128, 1], BF16, tag="wb1")
            nc.gpsimd.tensor_copy(wb1[:], xb[:, 0:1])


            mutis_by_c = {}  # final h per chain-dir, [128, HC*TPC] bf16
            prods_by_c = {}

            # largest windows first: their latency-bound steps overlap with
            # the later (smaller) windows' G matmuls and weight DMA
            for wi in (3, 2, 1, 0):
                w = WINDOWS[wi]
                half = w // 2
                gs = []
                whhs = []
                for d in (0, 1):
                    c = wi * 2 + d
                    wih, whh, whh2 = (prefetched.pop(c) if c in prefetched
                                      else load_chain_weights(c))
                    whhs.append((whh, whh2))
                    # ---- G + h state tile: [128, GC*TH + HC*TPC] fp8 ----
                    g = g_pool.tile([128, GTOT], FP8, tag="g")
                    wih3 = wih[:].rearrange("p (k n) -> p k n", k=DC)
                    xt3 = xt[:].rearrange("p (k t) -> p k t", t=TH)
                    for j in range(GC):
                        ps = psg_pool.tile([128, TH], F32, tag="gps")
                        for kp in range(DC // 2):
                            nc.tensor.matmul(
                                ps[:],
                                lhsT=wih3[:, 2 * kp:2 * kp + 2,
                                          j * 128:(j + 1) * 128],
                                rhs=xt3[:, 2 * kp:2 * kp + 2, :],
                                start=(kp == 0),
                                stop=(kp == DC // 2 - 1),
                                perf_mode=DR,
                            )
                        # eviction + bias fold (per-partition scalar per chunk)
                        nc.vector.tensor_scalar(
                            g[:, j * TH:(j + 1) * TH], ps[:],
                            biasr[:, c * GC + j:c * GC + j + 1], None,
                            ALU.add,
                        )
                    # invalid halo token columns -> EDGE (gates saturate)
                    g3 = g[:, 0:GLEN].rearrange("p (j t) -> p j t", t=TH)
                    nc.vector.copy_predicated(
                        g3[:, :, 0:4],
                        pgl[:].rearrange("p (j e) -> p j e", e=4),
                        negt[:].rearrange("p (j e) -> p j e", e=4),
                    )
                    nc.vector.copy_predicated(
                        g3[:, :, TH - 4:TH],
                        pgr[:].rearrange("p (j e) -> p j e", e=4),
                        negt[:].rearrange("p (j e) -> p j e", e=4),
                    )
                    gs.append(g)

                # ---- run both directions' chains, step-interleaved ----
                # c state: rebound each step (unconditioned updates);
                # h state lives inside each direction's g tile
                cst = [None, None]
                snaps = []
                for d in (0, 1):
                    sn = snap_pool.tile([128, HC * 4], BF16, tag="snap",
                                        name=f"snap{wi}_{d}")
                    snaps.append(sn)
                mut = [
                    muti_pool.tile([128, HC * TPC], BF16, tag="muti",
                                   name=f"muti{wi}_{d}")
                    for d in (0, 1)
                ]

                for t in range(w):
                    for d in (0, 1):
                        o = (t - half) if d == 0 else (half - t)
                        g = gs[d]
                        whh, whh2 = whhs[d]
                        g3 = g[:, 0:GLEN].rearrange("p (j t) -> p j t", t=TH)
                        if t == 0:
                            # gates are exactly the (bias-folded) G slice:
                            # ACT reads it straight from SBUF, no inject
                            src_sig = g3[:, 0:9, HALO + o:HALO + o + TPC]
                            src_tanh = g3[:, 9:12, HALO + o:HALO + o + TPC]
                        else:
                            gps = ps_pool.tile([128, 4 * H], F32,
                                               tag=f"gates{d}")
                            src_sig = gps[:, 0:3 * H].rearrange(
                                "p (c t) -> p c t", t=TPC)
                            src_tanh = gps[:, 3 * H:4 * H].rearrange(
                                "p (c t) -> p c t", t=TPC)
                            for nb in range(3):
                                nc.tensor.matmul(
                                    gps[:, nb * 512:(nb + 1) * 512],
                                    lhsT=id8[:],
                                    rhs=g3[:, nb * 4:(nb + 1) * 4,
                                           HALO + o:HALO + o + TPC],
                                    start=True,
                                    stop=False,
                                )
                        if t > 0:
                            # gates = G slice + W_hh @ h, all fp8 DoubleRow:
                            # k-pair (0,1) then the fused (id8,whh_k2) pair
                            # contracting (G_slice, h_k2)
                            whh3 = whh[:].rearrange("p (k n) -> p k n", k=2)
                            h3 = g[:, HOFF:GTOT].rearrange(
                                "p (k t) -> p k t", k=HC)
                            for j in range(GC):
                                nc.tensor.matmul(
                                    gps[:, j * 128:(j + 1) * 128],
                                    lhsT=whh3[:, 0:2, j * 128:(j + 1) * 128],
                                    rhs=h3[:, 0:2, :],
                                    start=False,
                                    stop=False,
                                    perf_mode=DR,
                                )
                                nc.tensor.matmul(
                                    gps[:, j * 128:(j + 1) * 128],
                                    lhsT=whh2[:, j * 256 + 128:
                                              (j + 1) * 256],
                                    rhs=g[:, HOFF + 2 * TPC:HOFF + 3 * TPC],
                                    start=False,
                                    stop=True,
                                )
                        # activations; gate order is [i,f,o,g] after host perm
                        sfo = tmp_pool.tile([128, 3 * H], BF16, tag="sfo")
                        nc.scalar.activation(
                            sfo[:].rearrange("p (c t) -> p c t", t=TPC),
                            src_sig, AF.Sigmoid, scale=IWS)
                        tg = tmp_pool.tile([128, H], BF16, tag="tg")
                        nc.scalar.activation(
                            tg[:].rearrange("p (c t) -> p c t", t=TPC),
                            src_tanh, AF.Tanh, scale=IWS)
                        # c_new = sig(f)*c + sig(i)*tanh(g)
                        cn = st_pool.tile([128, H], BF16, tag="cn",
                                          name=f"c{wi}_{d}_{t}")
                        if t > 0:
                            fc = tmp_pool.tile([128, H], BF16, tag="fc")
                            nc.vector.tensor_tensor(
                                fc[:], sfo[:, H:2 * H], cst[d][:], ALU.mult)
                            ig = tmp_pool.tile([128, H], BF16, tag="ig")
                            nc.vector.tensor_tensor(
                                ig[:], sfo[:, 0:H], tg[:], ALU.mult)
                            nc.vector.tensor_tensor(cn[:], ig[:], fc[:],
                                                    ALU.add)
                        else:
                            nc.vector.tensor_tensor(cn[:], sfo[:, 0:H], tg[:],
                                                    ALU.mult)
                        cst[d] = cn
                        tcn = tmp_pool.tile([128, H], BF16, tag="tcn")
                        nc.scalar.activation(tcn[:], cn[:], AF.Tanh)
                        # h = sig(o) * tanh(c); last step lands in bf16 muti
                        if t == w - 1:
                            nc.vector.tensor_tensor(
                                mut[d][:], sfo[:, 2 * H:3 * H], tcn[:],
                                ALU.mult)
                        else:
                            nc.vector.tensor_tensor(
                                g[:, HOFF:GTOT], sfo[:, 2 * H:3 * H], tcn[:],
                                ALU.mult)
                            # trailing-edge snapshot: one token column whose
                            # last valid step is t (real only on edge cores;
                            # merged predicated below)
                            if half <= t:
                                if d == 0:
                                    tok = TPC - 1 - (t - half)
                                    slot = tok - (TPC - 4)
                                else:
                                    tok = t - half
                                    slot = tok
                                h3n = g[:, HOFF:GTOT].rearrange(
                                    "p (k t) -> p k t", k=HC)
                                sn3 = snaps[d][:].rearrange(
                                    "p (k s) -> p k s", s=4)
                                nc.vector.tensor_copy(
                                    sn3[:, :, slot:slot + 1],
                                    h3n[:, :, tok:tok + 1])
                # merge trailing-edge snapshots into the final h
                for d in (0, 1):
                    m3 = mut[d][:].rearrange("p (k t) -> p k t", k=HC)
                    pm = (pmf if d == 0 else pmb)[:].rearrange(
                        "p (w k s) -> p w k s", w=NW, s=4)
                    sn3 = snaps[d][:].rearrange("p (k s) -> p k s", s=4)
                    cols = m3[:, :, TPC - 4:TPC] if d == 0 else m3[:, :, 0:4]
                    nc.vector.copy_predicated(cols, pm[:, wi], sn3[:])
                mutis_by_c[wi * 2] = mut[0]
                mutis_by_c[wi * 2 + 1] = mut[1]
                # attention products for this window (hides under later
                # windows' steps)
                xb3 = xb[:].rearrange("p (k t) -> p k t", t=TPC)
                for d in (0, 1):
                    pr = tmp_pool.tile([128, HC * TPC], BF16, tag=f"pr{d}",
                                       name=f"pr{wi}_{d}", bufs=NW)
                    nc.vector.tensor_tensor(
                        pr[:], mut[d][:], xb3[:, d * HC:(d + 1) * HC, :],
                        ALU.mult)
                    prods_by_c[wi * 2 + d] = pr
            mutis = [mutis_by_c[c] for c in range(NCH)]

            # ---- attention over the 4 window features ----
            xb3 = xb[:].rearrange("p (k t) -> p k t", t=TPC)
            score_ps = psg_pool.tile([128, NW], F32, tag="gps")
            prods = [prods_by_c[c] for c in range(NCH)]
            for wi in range(NW):
                for ci in range(2 * HC):
                    pr = prods[wi * 2 + ci // HC]
                    k = ci % HC
                    nc.tensor.matmul(
                        score_ps[:, wi:wi + 1],
                        lhsT=pr[:, k * TPC:(k + 1) * TPC],
                        rhs=ones[:],
                        start=(ci == 0),
                        stop=(ci == 2 * HC - 1),
                    )
            # softmax over the NW axis (token-partition [128, 4])
            mx = tmp_pool.tile([128, 1], F32, tag="mx")
            nc.vector.reduce_max(mx[:], score_ps[:], axis=AX.X)
            mxn = tmp_pool.tile([128, 1], F32, tag="mxn")
            nc.vector.tensor_scalar(mxn[:], mx[:], float(-SCALE), None,
                                    ALU.mult)
            ex = tmp_pool.tile([128, NW], F32, tag="ex")
            nc.scalar.activation(ex[:], score_ps[:], AF.Exp, bias=mxn[:],
                                 scale=float(SCALE))
            sm = tmp_pool.tile([128, 1], F32, tag="sm")
            nc.vector.reduce_sum(sm[:], ex[:], axis=AX.X)
            rs = tmp_pool.tile([128, 1], F32, tag="rs")
            nc.vector.reciprocal(rs[:], sm[:])
            attn = tmp_pool.tile([128, NW], BF16, tag="attn")
            nc.vector.tensor_scalar(attn[:], ex[:], rs[:], None, ALU.mult)
            # per-window: transpose attn column to [1,128], replicate to
            # [1,384], outer-product with ones to broadcast over partitions
            bcs = []
            for wi in range(NW):
                at_ps = psg_pool.tile([1, TPC], BF16, tag="gps",
                                      name=f"atps{wi}")
                nc.tensor.transpose(at_ps[:], attn[:, wi:wi + 1], idb[:])
                at_sb = tmp_pool.tile([1, HC * TPC], BF16, tag="atsb",
                                      name=f"atsb{wi}")
                for k in range(HC):
                    nc.vector.tensor_copy(at_sb[:, k * TPC:(k + 1) * TPC],
                                          at_ps[:])
                bc_ps = psg_pool.tile([128, HC * TPC], F32, tag="gps",
                                      name=f"bcps{wi}")
                nc.tensor.matmul(
                    bc_ps[:], lhsT=onesr[:, 0:128], rhs=at_sb[:],
                    start=True, stop=True,
                )
                bc = tmp_pool.tile([128, HC * TPC], BF16, tag="bc",
                                   name=f"bc{wi}", bufs=NW)
                nc.vector.tensor_copy(bc[:], bc_ps[:])
                bcs.append(bc)
            accs = []
            for d in (0, 1):
                eng = nc.vector if d == 0 else nc.gpsimd
                acc = fin_pool.tile([128, HC * TPC], BF16, tag=f"acc{d}")
                t1 = tmp_pool.tile([128, HC * TPC], BF16, tag=f"t1{d}")
                eng.tensor_tensor(t1[:], mutis[d][:], bcs[0][:], ALU.mult)
                for wi in range(1, NW):
                    t2 = tmp_pool.tile([128, HC * TPC], BF16, tag=f"t2{d}")
                    eng.tensor_tensor(
                        t2[:], mutis[wi * 2 + d][:], bcs[wi][:], ALU.mult)
                    eng.tensor_tensor(
                        t1[:] if wi < NW - 1 else acc[:], t1[:], t2[:],
                        ALU.add)
                # residual: out = x + local_feat
                eng.tensor_tensor(
                    acc[:], acc[:], xb3[:, d * HC:(d + 1) * HC, :], ALU.add)
                accs.append(acc)
            # ---- linear head: logits [9, 128] ----
            lg_ps = psg_pool.tile([NL, TPC], F32, tag="gps")
            for ci in range(DC):
                d = ci // HC
                k = ci % HC
                nc.tensor.matmul(
                    lg_ps[:],
                    lhsT=linw[:, ci * NL:(ci + 1) * NL],
                    rhs=accs[d][:, k * TPC:(k + 1) * TPC],
                    start=(ci == 0),
                    stop=False,
                )
            nc.tensor.matmul(
                lg_ps[:], lhsT=linb[:], rhs=onesr[:, 0:TPC],
                start=False, stop=True,
            )
            ob = fin_pool.tile([NL, TPC], F32, tag="ob")
            nc.vector.tensor_copy(ob[:], lg_ps[:])
            nc.sync.dma_start(out=out_d[:], in_=ob[:])

    nc.finalize()
    return nc


def _valid_scatter_np(x, valid_ids):
    Bx, Lx, Dx = x.shape
    v = (valid_ids == 1)
    out = np.zeros_like(x)
    for b in range(Bx):
        sel = x[b][v[b]]
        out[b, :sel.shape[0]] = sel
    return out


def _to_fp8(a):
    f8 = mybir.dt.np(FP8)  # ml_dtypes.float8_e4m3 (TRN flavor, max 240)
    return np.clip(np.asarray(a, np.float32), -240.0, 240.0).astype(f8)


def _host_prep(inputs):
    seq_out = np.asarray(inputs["seq_out"], np.float32)
    valid_ids = np.asarray(inputs["valid_ids"])
    x = _valid_scatter_np(seq_out, valid_ids)  # [B,L,D] f32

    bf = ml_dtypes.bfloat16
    f8 = mybir.dt.np(FP8)
    # gate permutation [i,f,g,o] -> [i,f,o,g]
    perm = np.concatenate([
        np.arange(0, H), np.arange(H, 2 * H),
        np.arange(3 * H, 4 * H), np.arange(2 * H, 3 * H),
    ])
    # weights, chain order c = window_idx*2 + dir (0=f, 1=b)
    wih = np.empty((NCH, D, 4 * H), f8)
    whh = np.empty((NCH, H, 4 * H), f8)
    whh2 = np.empty((NCH, 128, GC * 2 * 128), f8)
    id8v = np.eye(128, dtype=f8)
    biasv = np.empty((128, NCH * GC), np.float32)
    for wi in range(NW):
        for d, sfx in ((0, "f"), (1, "b")):
            c = wi * 2 + d
            wih[c] = _to_fp8(
                WS * np.asarray(inputs[f"w_ih_{sfx}"][wi], np.float32)[perm].T)
            whh[c] = _to_fp8(
                WS * np.asarray(inputs[f"w_hh_{sfx}"][wi], np.float32)[perm].T)
            wk2 = np.asarray(whh[c][2 * 128:3 * 128], f8)  # [128, 4H]
            for j in range(GC):
                whh2[c, :, j * 256:j * 256 + 128] = id8v
                whh2[c, :, j * 256 + 128:(j + 1) * 256] = \
                    wk2[:, j * 128:(j + 1) * 128]
            bv = (np.asarray(inputs[f"b_ih_{sfx}"][wi], np.float32)
                  + np.asarray(inputs[f"b_hh_{sfx}"][wi], np.float32))[perm]
            biasv[:, c * GC:(c + 1) * GC] = WS * bv.reshape(GC, 128).T
    linw = np.asarray(inputs["lin_w"], np.float32).T.astype(bf)  # [768, 9]
    linb = np.asarray(inputs["lin_b"], np.float32)[None, :].astype(bf)
    id8 = np.eye(128, dtype=f8)
    idb = np.eye(128, dtype=bf)
    ones = np.ones((128, 1), bf)
    onesr = np.ones((1, 512), bf)

    in_maps = []
    for core in range(NCORES):
        b = core // 2
        right = core % 2  # 0: row-start half, 1: row-end half
        t0 = right * TPC
        # halo slice [t0-4, t0+132) of row b, zero-padded outside [0, L)
        xh = np.zeros((TH, D), np.float32)
        lo = max(0, t0 - HALO)
        hi = min(L, t0 + TPC + HALO)
        xh[lo - (t0 - HALO):hi - (t0 - HALO)] = x[b, lo:hi]
        xt = _to_fp8(np.ascontiguousarray(xh.T))              # [768, 136]
        xbc = np.ascontiguousarray(x[b, t0:t0 + TPC].T).astype(bf)
        # G-edge preds: invalid halo token columns (per chunk, 4 cols)
        pgl = np.full((128, GC * 4), 0 if right else 1, np.uint8)
        pgr = np.full((128, GC * 4), 1 if right else 0, np.uint8)
        # muti merge preds: per window, 1s on the `half` trailing-edge slots
        pmf = np.zeros((128, NW * HC * 4), np.uint8)
        pmb = np.zeros((128, NW * HC * 4), np.uint8)
        for wi, w in enumerate(WINDOWS):
            half = w // 2
            for k in range(HC):
                base = (wi * HC + k) * 4
                if right:  # fwd trailing at row end: slots 4-half..3
                    pmf[:, base + 4 - half:base + 4] = 1
                else:      # bwd trailing at row start: slots 0..half-1
                    pmb[:, base:base + half] = 1
        in_maps.append({
            "xt": xt, "xb": xbc,
            "wih": wih, "whh": whh, "whh2": whh2, "bias": biasv,
            "linw": linw, "linb": linb,
            "id8": id8, "idb": idb, "ones": ones, "onesr": onesr,
            "pgl": pgl, "pgr": pgr, "pmf": pmf, "pmb": pmb,
        })
    return in_maps


def kernel(**inputs) -> np.ndarray:
    if "nc" not in _CACHE:
        _CACHE["nc"] = _build()
    nc = _CACHE["nc"]
    in_maps = _host_prep(inputs)
    res = run_bass_kernel_spmd(nc, in_maps, core_ids=list(range(NCORES)))
    out = np.empty((B, L, NL), np.float32)
    for core in range(NCORES):
        b = core // 2
        t0 = (core % 2) * TPC
        out[b, t0:t0 + TPC] = res.results[core]["out"].T
    return out


# revision 44
# speedup vs baseline: 1.0107x; 1.0107x over previous
"""Trainium2 Bass kernel for nn_HGNER (windowed bi-LSTM + attention + linear head).

Sharding: 8 cores x 128 tokens (data-parallel over the flattened (B,L) token
axis; each core gets half of one batch row plus a 4-token halo). Small LSTM /
linear params are replicated to every core.

Layout inside a core: "feature-partition" — SBUF partitions carry a 128-wide
feature chunk, the free dim packs (chunk, token). This makes the recurrent
h @ W_hh matmuls transpose-free and keeps elementwise ops on long free dims.

Perf structure vs the straightforward version:
 - All heavy matmuls run in fp8-e4m3 with DoubleRow perf mode (2 contraction
   rows per PE pass). Weights are pre-scaled by 16 on the host (to dodge fp8
   subnormals); the 1/16 descale rides the ACT engine's free input scale.
 - Gate order is host-permuted to [i,f,o,g] so each step needs one sigmoid
   pass (3H wide) + one tanh pass instead of three activation calls.
 - The per-step state masking (ragged window edges) is replaced by: (a) G
   columns at invalid halo positions forced to -224 (=> sigma,tanh saturate,
   state becomes 0 which matches "keep zero state" on the leading side), and
   (b) for the trailing side, per-step single-token-column snapshots of h
   plus a tiny 4-column predicated merge at chain end.
 - G's PSUM->SBUF eviction is a DVE tensor_scalar that also folds in the
   LSTM bias as a per-partition scalar (no bias matmuls).
 - Windows run largest-first so the small windows' G matmuls and weight DMA
   fill the tensor-engine gaps of the latency-bound big-window steps; the
   first window's weights are DMA'd ahead of all non-critical constants.
"""

import numpy as np
import ml_dtypes

import bass_rust
import concourse.bass as bass
import concourse.bacc as bacc_mod
import concourse.mybir as mybir
from concourse.tile import TileContext
from concourse.bass_utils import run_bass_kernel_spmd

F32 = mybir.dt.float32
BF16 = mybir.dt.bfloat16
FP8 = mybir.dt.float8e4
U8 = mybir.dt.uint8
AF = mybir.ActivationFunctionType
ALU = mybir.AluOpType
AX = mybir.AxisListType
DR = mybir.MatmulPerfMode.DoubleRow

B, L, D, H, NW, NL = 4, 256, 768, 384, 4, 9
WINDOWS = (3, 5, 7, 9)
NCORES = 8
TPC = 128          # tokens per core
HALO = 4           # max half-window
TH = TPC + 2 * HALO  # 136 tokens incl. halo
DC = D // 128      # 6 chunks of input features
HC = H // 128      # 3 chunks of hidden features
GC = 4 * H // 128  # 12 chunks of gate features
NCH = 2 * NW       # 8 (window, direction) chains
GLEN = GC * TH     # G region length in the per-chain state tile
HOFF = GLEN        # h region offset (h lives beside G so one DoubleRow
                   # matmul can contract (id8, whh_k2) x (G_slice, h_k2))
GTOT = GLEN + HC * TPC
SCALE = 1.0 / np.sqrt(np.float32(D))
WS = 16.0          # host-side weight scale (fp8 subnormal avoidance)
IWS = 1.0 / WS
EDGE = -224.0      # fp8-storable; /16 => sigma(-14)~8e-7, tanh(-14)~-1

FUSE = set()  # fused G-inject DoubleRow path: disabled — repeated
# custom-AP DR rounds on a reused PSUM tile fault the device (repro'd
# in isolation; single rounds work). Keep empty.

_CACHE = {}


def _build():
    nc = bacc_mod.Bacc()

    # ---- DRAM I/O ----
    xt_d = nc.dram_tensor("xt", [D, TH], FP8, kind="ExternalInput")
    xb_d = nc.dram_tensor("xb", [D, TPC], BF16, kind="ExternalInput")
    wih_d = nc.dram_tensor("wih", [NCH, D, 4 * H], FP8, kind="ExternalInput")
    whh_d = nc.dram_tensor("whh", [NCH, H, 4 * H], FP8, kind="ExternalInput")
    # per chain: [id8 | whh_k2] plane-pairs per gate chunk, for the fused
    # (G-inject + 3rd h-chunk) DoubleRow matmul
    whh2_d = nc.dram_tensor("whh2", [NCH, 128, GC * 2 * 128], FP8,
                            kind="ExternalInput")
    bias_d = nc.dram_tensor("bias", [128, NCH * GC], F32, kind="ExternalInput")
    linw_d = nc.dram_tensor("linw", [D, NL], BF16, kind="ExternalInput")
    linb_d = nc.dram_tensor("linb", [1, NL], BF16, kind="ExternalInput")
    id8_d = nc.dram_tensor("id8", [128, 128], FP8, kind="ExternalInput")
    idb_d = nc.dram_tensor("idb", [128, 128], BF16, kind="ExternalInput")
    ones_d = nc.dram_tensor("ones", [128, 1], BF16, kind="ExternalInput")
    onesr_d = nc.dram_tensor("onesr", [1, 512], BF16, kind="ExternalInput")
    # edge predication masks (per-core data; program is identical on all cores)
    pgl_d = nc.dram_tensor("pgl", [128, GC * 4], U8, kind="ExternalInput")
    pgr_d = nc.dram_tensor("pgr", [128, GC * 4], U8, kind="ExternalInput")
    pmf_d = nc.dram_tensor("pmf", [128, NW * HC * 4], U8, kind="ExternalInput")
    pmb_d = nc.dram_tensor("pmb", [128, NW * HC * 4], U8, kind="ExternalInput")
    out_d = nc.dram_tensor("out", [NL, TPC], F32, kind="ExternalOutput")

    with TileContext(nc) as tc:
        with (
            tc.tile_pool(name="const", bufs=1) as cpool,
            tc.tile_pool(name="wih", bufs=2) as wih_pool,
            tc.tile_pool(name="whh", bufs=6) as whh_pool,
            tc.tile_pool(name="g", bufs=8) as g_pool,
            tc.tile_pool(name="muti", bufs=NCH + 2) as muti_pool,
            tc.tile_pool(name="st", bufs=6) as st_pool,
            tc.tile_pool(name="snap", bufs=4) as snap_pool,
            tc.tile_pool(name="tmp", bufs=8) as tmp_pool,
            tc.tile_pool(name="fin", bufs=2) as fin_pool,
            tc.tile_pool(name="ps", bufs=1, space="PSUM") as ps_pool,
            tc.tile_pool(name="psg", bufs=2, space="PSUM") as psg_pool,
        ):
            def load_chain_weights(c):
                wihT = wih_pool.tile([128, DC * 4 * H], FP8, tag="wih")
                nc.sync.dma_start(
                    out=wihT[:].rearrange("p (k n) -> p k n", k=DC),
                    in_=wih_d[c].rearrange("(k p) n -> p k n", p=128),
                )
                whhT = whh_pool.tile([128, 2 * 4 * H], FP8, tag="whh")
                nc.sync.dma_start(
                    out=whhT[:].rearrange("p (k n) -> p k n", k=2),
                    in_=whh_d[c, 0:2 * 128].rearrange(
                        "(k p) n -> p k n", p=128),
                )
                whh2T = whh_pool.tile([128, GC * 2 * 128], FP8, tag="whh2")
                nc.sync.dma_start(out=whh2T[:], in_=whh2_d[c])
                return (wihT, whhT, whh2T)

            # ---- critical-path DMAs first: x, first window's weights ----
            xt = cpool.tile([128, DC * TH], FP8, tag="xt")
            nc.sync.dma_start(
                out=xt[:].rearrange("p (k t) -> p k t", t=TH),
                in_=xt_d[:].rearrange("(k p) t -> p k t", p=128),
            )
            biasr = cpool.tile([128, NCH * GC], F32, tag="bias")
            nc.sync.dma_start(out=biasr[:], in_=bias_d[:])
            prefetched = {6: load_chain_weights(6), 7: load_chain_weights(7)}
            pgl = cpool.tile([128, GC * 4], U8, tag="pgl")
            nc.sync.dma_start(out=pgl[:], in_=pgl_d[:])
            pgr = cpool.tile([128, GC * 4], U8, tag="pgr")
            nc.sync.dma_start(out=pgr[:], in_=pgr_d[:])
            id8 = cpool.tile([128, 128], FP8, tag="id8")
            nc.sync.dma_start(out=id8[:], in_=id8_d[:])
            # ---- non-critical consts (epilogue / later windows) ----
            xb = cpool.tile([128, DC * TPC], BF16, tag="xb")
            nc.sync.dma_start(
                out=xb[:].rearrange("p (k t) -> p k t", t=TPC),
                in_=xb_d[:].rearrange("(k p) t -> p k t", p=128),
            )
            linw = cpool.tile([128, DC * NL], BF16, tag="linw")
            nc.sync.dma_start(
                out=linw[:].rearrange("p (k n) -> p k n", n=NL),
                in_=linw_d[:].rearrange("(k p) n -> p k n", p=128),
            )
            linb = cpool.tile([1, NL], BF16, tag="linb")
            nc.sync.dma_start(out=linb[:], in_=linb_d[:])
            idb = cpool.tile([128, 128], BF16, tag="idb")
            nc.sync.dma_start(out=idb[:], in_=idb_d[:])
            ones = cpool.tile([128, 1], BF16, tag="ones")
            nc.sync.dma_start(out=ones[:], in_=ones_d[:])
            onesr = cpool.tile([1, 512], BF16, tag="onesr")
            nc.sync.dma_start(out=onesr[:], in_=onesr_d[:])
            pmf = cpool.tile([128, NW * HC * 4], U8, tag="pmf")
            nc.sync.dma_start(out=pmf[:], in_=pmf_d[:])
            pmb = cpool.tile([128, NW * HC * 4], U8, tag="pmb")
            nc.sync.dma_start(out=pmb[:], in_=pmb_d[:])
            negt = cpool.tile([128, GC * 4], FP8, tag="negt")
            nc.vector.memset(negt[:], EDGE)

            # one-time engine touches of DMA-loaded consts: collapse later
            # waits to a single semaphore (instr structs have 1 wait slot)
            wu8 = cpool.tile([128, 1], U8, tag="wu8")
            nc.vector.tensor_copy(wu8[:], pgl[:, 0:1])
            nc.vector.tensor_copy(wu8[:], pgr[:, 0:1])
            nc.vector.tensor_copy(wu8[:], pmf[:, 0:1])
            nc.vector.tensor_copy(wu8[:], pmb[:, 0:1])
            wb0 = cpool.tile([128, 1], BF16, tag="wb0")
            nc.vector.tensor_copy(wb0[:], xb[:, 0:1])
            wb1 = cpool.tile([128, 1], BF16, tag="wb1")
            nc.gpsimd.tensor_copy(wb1[:], xb[:, 0:1])


            mutis_by_c = {}  # final h per chain-dir, [128, HC*TPC] bf16
            prods_by_c = {}

            # largest windows first: their latency-bound steps overlap with
            # the later (smaller) windows' G matmuls and weight DMA
            for wi in (3, 2, 1, 0):
                w = WINDOWS[wi]
                half = w // 2
                gs = []
                whhs = []
                for d in (0, 1):
                    c = wi * 2 + d
                    wih, whh, whh2 = (prefetched.pop(c) if c in prefetched
                                      else load_chain_weights(c))
                    whhs.append((whh, whh2))
                    # ---- G + h state tile: [128, GC*TH + HC*TPC] fp8 ----
                    g = g_pool.tile([128, GTOT], FP8, tag="g")
                    wih3 = wih[:].rearrange("p (k n) -> p k n", k=DC)
                    xt3 = xt[:].rearrange("p (k t) -> p k t", t=TH)
                    for j in range(GC):
                        ps = psg_pool.tile([128, TH], F32, tag="gps")
                        for kp in range(DC // 2):
                            nc.tensor.matmul(
                                ps[:],
                                lhsT=wih3[:, 2 * kp:2 * kp + 2,
                                          j * 128:(j + 1) * 128],
                                rhs=xt3[:, 2 * kp:2 * kp + 2, :],
                                start=(kp == 0),
                                stop=(kp == DC // 2 - 1),
                                perf_mode=DR,
                            )
                        # eviction + bias fold (per-partition scalar per chunk)
                        nc.vector.tensor_scalar(
                            g[:, j * TH:(j + 1) * TH], ps[:],
                            biasr[:, c * GC + j:c * GC + j + 1], None,
                            ALU.add,
                        )
                    # invalid halo token columns -> EDGE (gates saturate)
                    g3 = g[:, 0:GLEN].rearrange("p (j t) -> p j t", t=TH)
                    nc.vector.copy_predicated(
                        g3[:, :, 0:4],
                        pgl[:].rearrange("p (j e) -> p j e", e=4),
                        negt[:].rearrange("p (j e) -> p j e", e=4),
                    )
                    nc.vector.copy_predicated(
                        g3[:, :, TH - 4:TH],
                        pgr[:].rearrange("p (j e) -> p j e", e=4),
                        negt[:].rearrange("p (j e) -> p j e", e=4),
                    )
                    gs.append(g)

                # ---- run both directions' chains, step-interleaved ----
                # c state: rebound each step (unconditioned updates);
                # h state lives inside each direction's g tile
                cst = [None, None]
                snaps = []
                for d in (0, 1):
                    sn = snap_pool.tile([128, HC * 4], BF16, tag="snap",
                                        name=f"snap{wi}_{d}")
                    snaps.append(sn)
                mut = [
                    muti_pool.tile([128, HC * TPC], BF16, tag="muti",
                                   name=f"muti{wi}_{d}")
                    for d in (0, 1)
                ]

                for t in range(w):
                    for d in (0, 1):
                        o = (t - half) if d == 0 else (half - t)
                        g = gs[d]
                        whh, whh2 = whhs[d]
                        g3 = g[:, 0:GLEN].rearrange("p (j t) -> p j t", t=TH)
                        if t == 0:
                            # gates are exactly the (bias-folded) G slice:
                            # ACT reads it straight from SBUF, no inject
                            src_sig = g3[:, 0:9, HALO + o:HALO + o + TPC]
                            src_tanh = g3[:, 9:12, HALO + o:HALO + o + TPC]
                        else:
                            gps = ps_pool.tile([128, 4 * H], F32,
                                               tag=f"gates{d}")
                            src_sig = gps[:, 0:3 * H].rearrange(
                                "p (c t) -> p c t", t=TPC)
                            src_tanh = gps[:, 3 * H:4 * H].rearrange(
                                "p (c t) -> p c t", t=TPC)
                            if wi not in FUSE:
                                for nb in range(3):
                                    nc.tensor.matmul(
                                        gps[:, nb * 512:(nb + 1) * 512],
                                        lhsT=id8[:],
                                        rhs=g3[:, nb * 4:(nb + 1) * 4,
                                               HALO + o:HALO + o + TPC],
                                        start=True,
                                        stop=False,
                                    )
                        if t > 0:
                            # gates = G slice + W_hh @ h, all fp8 DoubleRow:
                            # k-pair (0,1) then the fused (id8,whh_k2) pair
                            # contracting (G_slice, h_k2)
                            whh3 = whh[:].rearrange("p (k n) -> p k n", k=2)
                            h3 = g[:, HOFF:GTOT].rearrange(
                                "p (k t) -> p k t", k=HC)
                            for j in range(GC):
                                nc.tensor.matmul(
                                    gps[:, j * 128:(j + 1) * 128],
                                    lhsT=whh3[:, 0:2, j * 128:(j + 1) * 128],
                                    rhs=h3[:, 0:2, :],
                                    # start clears the PSUM zero-region at
                                    # bank granularity: only the first chunk
                                    # of each 512-col bank may set it
                                    start=(wi in FUSE and j % 4 == 0),
                                    stop=False,
                                    perf_mode=DR,
                                )
                                if wi in FUSE:
                                    goff = j * TH + HALO + o
                                    rhs2 = g[:, goff:goff + TPC].copy()
                                    rhs2.ap = bass_rust.VecI64Pair(
                                        [[GTOT, 128],
                                         [HOFF + 2 * TPC - goff, 2],
                                         [1, TPC]])
                                    nc.tensor.matmul(
                                        gps[:, j * 128:(j + 1) * 128],
                                        lhsT=whh2[:, j * 256:(j + 1) * 256]
                                        .rearrange("p (two q) -> p two q",
                                                   two=2),
                                        rhs=rhs2,
                                        start=False,
                                        stop=(j % 4 == 3),
                                        perf_mode=DR,
                                    )
                                else:
                                    nc.tensor.matmul(
                                        gps[:, j * 128:(j + 1) * 128],
                                        lhsT=whh2[:, j * 256 + 128:
                                                  (j + 1) * 256],
                                        rhs=g[:, HOFF + 2 * TPC:HOFF + 3 * TPC],
                                        start=False,
                                        stop=True,
                                    )
                        # activations; gate order is [i,f,o,g] after host perm
                        sfo = tmp_pool.tile([128, 3 * H], BF16, tag="sfo")
                        nc.scalar.activation(
                            sfo[:].rearrange("p (c t) -> p c t", t=TPC),
                            src_sig, AF.Sigmoid, scale=IWS)
                        tg = tmp_pool.tile([128, H], BF16, tag="tg")
                        nc.scalar.activation(
                            tg[:].rearrange("p (c t) -> p c t", t=TPC),
                            src_tanh, AF.Tanh, scale=IWS)
                        # c_new = sig(f)*c + sig(i)*tanh(g)
                        cn = st_pool.tile([128, H], BF16, tag="cn",
                                          name=f"c{wi}_{d}_{t}")
                        if t > 0:
                            fc = tmp_pool.tile([128, H], BF16, tag="fc")
                            nc.vector.tensor_tensor(
                                fc[:], sfo[:, H:2 * H], cst[d][:], ALU.mult)
                            ig = tmp_pool.tile([128, H], BF16, tag="ig")
                            nc.vector.tensor_tensor(
                                ig[:], sfo[:, 0:H], tg[:], ALU.mult)
                            nc.vector.tensor_tensor(cn[:], ig[:], fc[:],
                                                    ALU.add)
                        else:
                            nc.vector.tensor_tensor(cn[:], sfo[:, 0:H], tg[:],
                                                    ALU.mult)
                        cst[d] = cn
                        tcn = tmp_pool.tile([128, H], BF16, tag="tcn")
                        nc.scalar.activation(tcn[:], cn[:], AF.Tanh)
                        # h = sig(o) * tanh(c); last step lands in bf16 muti
                        if t == w - 1:
                            nc.vector.tensor_tensor(
                                mut[d][:], sfo[:, 2 * H:3 * H], tcn[:],
                                ALU.mult)
                        else:
                            nc.vector.tensor_tensor(
                                g[:, HOFF:GTOT], sfo[:, 2 * H:3 * H], tcn[:],
                                ALU.mult)
                            # trailing-edge snapshot: one token column whose
                            # last valid step is t (real only on edge cores;
                            # merged predicated below)
                            if half <= t:
                                if d == 0:
                                    tok = TPC - 1 - (t - half)
                                    slot = tok - (TPC - 4)
                                else:
                                    tok = t - half
                                    slot = tok
                                h3n = g[:, HOFF:GTOT].rearrange(
                                    "p (k t) -> p k t", k=HC)
                                sn3 = snaps[d][:].rearrange(
                                    "p (k s) -> p k s", s=4)
                                nc.vector.tensor_copy(
                                    sn3[:, :, slot:slot + 1],
                                    h3n[:, :, tok:tok + 1])
                # merge trailing-edge snapshots into the final h
                for d in (0, 1):
                    m3 = mut[d][:].rearrange("p (k t) -> p k t", k=HC)
                    pm = (pmf if d == 0 else pmb)[:].rearrange(
                        "p (w k s) -> p w k s", w=NW, s=4)
                    sn3 = snaps[d][:].rearrange("p (k s) -> p k s", s=4)
                    cols = m3[:, :, TPC - 4:TPC] if d == 0 else m3[:, :, 0:4]
                    nc.vector.copy_predicated(cols, pm[:, wi], sn3[:])
                mutis_by_c[wi * 2] = mut[0]
                mutis_by_c[wi * 2 + 1] = mut[1]
                # attention products for this window (hides under later
                # windows' steps)
                xb3 = xb[:].rearrange("p (k t) -> p k t", t=TPC)
                for d in (0, 1):
                    pr = tmp_pool.tile([128, HC * TPC], BF16, tag=f"pr{d}",
                                       name=f"pr{wi}_{d}", bufs=NW)
                    nc.vector.tensor_tensor(
                        pr[:], mut[d][:], xb3[:, d * HC:(d + 1) * HC, :],
                        ALU.mult)
                    prods_by_c[wi * 2 + d] = pr
            mutis = [mutis_by_c[c] for c in range(NCH)]

            # ---- attention over the 4 window features ----
            xb3 = xb[:].rearrange("p (k t) -> p k t", t=TPC)
            score_ps = psg_pool.tile([128, NW], F32, tag="gps")
            prods = [prods_by_c[c] for c in range(NCH)]
            for wi in range(NW):
                for ci in range(2 * HC):
                    pr = prods[wi * 2 + ci // HC]
                    k = ci % HC
                    nc.tensor.matmul(
                        score_ps[:, wi:wi + 1],
                        lhsT=pr[:, k * TPC:(k + 1) * TPC],
                        rhs=ones[:],
                        start=(ci == 0),
                        stop=(ci == 2 * HC - 1),
                    )
            # softmax over the NW axis (token-partition [128, 4])
            mx = tmp_pool.tile([128, 1], F32, tag="mx")
            nc.vector.reduce_max(mx[:], score_ps[:], axis=AX.X)
            mxn = tmp_pool.tile([128, 1], F32, tag="mxn")
            nc.vector.tensor_scalar(mxn[:], mx[:], float(-SCALE), None,
                                    ALU.mult)
            ex = tmp_pool.tile([128, NW], F32, tag="ex")
            nc.scalar.activation(ex[:], score_ps[:], AF.Exp, bias=mxn[:],
                                 scale=float(SCALE))
            sm = tmp_pool.tile([128, 1], F32, tag="sm")
            nc.vector.reduce_sum(sm[:], ex[:], axis=AX.X)
            rs = tmp_pool.tile([128, 1], F32, tag="rs")
            nc.vector.reciprocal(rs[:], sm[:])
            attn = tmp_pool.tile([128, NW], BF16, tag="attn")
            nc.vector.tensor_scalar(attn[:], ex[:], rs[:], None, ALU.mult)
            # per-window: transpose attn column to [1,128], replicate to
            # [1,384], outer-product with ones to broadcast over partitions
            bcs = []
            for wi in range(NW):
                at_ps = psg_pool.tile([1, TPC], BF16, tag="gps",
                                      name=f"atps{wi}")
                nc.tensor.transpose(at_ps[:], attn[:, wi:wi + 1], idb[:])
                at_sb = tmp_pool.tile([1, HC * TPC], BF16, tag="atsb",
                                      name=f"atsb{wi}")
                for k in range(HC):
                    nc.vector.tensor_copy(at_sb[:, k * TPC:(k + 1) * TPC],
                                          at_ps[:])
                bc_ps = psg_pool.tile([128, HC * TPC], F32, tag="gps",
                                      name=f"bcps{wi}")
                nc.tensor.matmul(
                    bc_ps[:], lhsT=onesr[:, 0:128], rhs=at_sb[:],
                    start=True, stop=True,
                )
                bc = tmp_pool.tile([128, HC * TPC], BF16, tag="bc",
                                   name=f"bc{wi}", bufs=NW)
                nc.vector.tensor_copy(bc[:], bc_ps[:])
                bcs.append(bc)
            accs = []
            for d in (0, 1):
                eng = nc.vector if d == 0 else nc.gpsimd
                acc = fin_pool.tile([128, HC * TPC], BF16, tag=f"acc{d}")
                t1 = tmp_pool.tile([128, HC * TPC], BF16, tag=f"t1{d}")
                eng.tensor_tensor(t1[:], mutis[d][:], bcs[0][:], ALU.mult)
                for wi in range(1, NW):
                    t2 = tmp_pool.tile([128, HC * TPC], BF16, tag=f"t2{d}")
                    eng.tensor_tensor(
                        t2[:], mutis[wi * 2 + d][:], bcs[wi][:], ALU.mult)
                    eng.tensor_tensor(
                        t1[:] if wi < NW - 1 else acc[:], t1[:], t2[:],
                        ALU.add)
                # residual: out = x + local_feat
                eng.tensor_tensor(
                    acc[:], acc[:], xb3[:, d * HC:(d + 1) * HC, :], ALU.add)
                accs.append(acc)
            # ---- linear head: logits [9, 128] ----
            lg_ps = psg_pool.tile([NL, TPC], F32, tag="gps")
            for ci in range(DC):
                d = ci // HC
                k = ci % HC
                nc.tensor.matmul(
                    lg_ps[:],
                    lhsT=linw[:, ci * NL:(ci + 1) * NL],
                    rhs=accs[d][:, k * TPC:(k + 1) * TPC],
                    start=(ci == 0),
                    stop=False,
                )
            nc.tensor.matmul(
                lg_ps[:], lhsT=linb[:], rhs=onesr[:, 0:TPC],
                start=False, stop=True,
            )
            ob = fin_pool.tile([NL, TPC], F32, tag="ob")
            nc.vector.tensor_copy(ob[:], lg_ps[:])
            nc.sync.dma_start(out=out_d[:], in_=ob[:])

    nc.finalize()
    return nc


def _valid_scatter_np(x, valid_ids):
    Bx, Lx, Dx = x.shape
    v = (valid_ids == 1)
    out = np.zeros_like(x)
    for b in range(Bx):
        sel = x[b][v[b]]
        out[b, :sel.shape[0]] = sel
    return out


def _to_fp8(a):
    f8 = mybir.dt.np(FP8)  # ml_dtypes.float8_e4m3 (TRN flavor, max 240)
    return np.clip(np.asarray(a, np.float32), -240.0, 240.0).astype(f8)


def _host_prep(inputs):
    seq_out = np.asarray(inputs["seq_out"], np.float32)
    valid_ids = np.asarray(inputs["valid_ids"])
    x = _valid_scatter_np(seq_out, valid_ids)  # [B,L,D] f32

    bf = ml_dtypes.bfloat16
    f8 = mybir.dt.np(FP8)
    # gate permutation [i,f,g,o] -> [i,f,o,g]
    perm = np.concatenate([
        np.arange(0, H), np.arange(H, 2 * H),
        np.arange(3 * H, 4 * H), np.arange(2 * H, 3 * H),
    ])
    # weights, chain order c = window_idx*2 + dir (0=f, 1=b)
    wih = np.empty((NCH, D, 4 * H), f8)
    whh = np.empty((NCH, H, 4 * H), f8)
    whh2 = np.empty((NCH, 128, GC * 2 * 128), f8)
    id8v = np.eye(128, dtype=f8)
    biasv = np.empty((128, NCH * GC), np.float32)
    for wi in range(NW):
        for d, sfx in ((0, "f"), (1, "b")):
            c = wi * 2 + d
            wih[c] = _to_fp8(
                WS * np.asarray(inputs[f"w_ih_{sfx}"][wi], np.float32)[perm].T)
            whh[c] = _to_fp8(
                WS * np.asarray(inputs[f"w_hh_{sfx}"][wi], np.float32)[perm].T)
            wk2 = np.asarray(whh[c][2 * 128:3 * 128], f8)  # [128, 4H]
            for j in range(GC):
                whh2[c, :, j * 256:j * 256 + 128] = id8v
                whh2[c, :, j * 256 + 128:(j + 1) * 256] = \
                    wk2[:, j * 128:(j + 1) * 128]
            bv = (np.asarray(inputs[f"b_ih_{sfx}"][wi], np.float32)
                  + np.asarray(inputs[f"b_hh_{sfx}"][wi], np.float32))[perm]
            biasv[:, c * GC:(c + 1) * GC] = WS * bv.reshape(GC, 128).T
    linw = np.asarray(inputs["lin_w"], np.float32).T.astype(bf)  # [768, 9]
    linb = np.asarray(inputs["lin_b"], np.float32)[None, :].astype(bf)
    id8 = np.eye(128, dtype=f8)
    idb = np.eye(128, dtype=bf)
    ones = np.ones((128, 1), bf)
    onesr = np.ones((1, 512), bf)

    in_maps = []
    for core in range(NCORES):
        b = core // 2
        right = core % 2  # 0: row-start half, 1: row-end half
        t0 = right * TPC
        # halo slice [t0-4, t0+132) of row b, zero-padded outside [0, L)
        xh = np.zeros((TH, D), np.float32)
        lo = max(0, t0 - HALO)
        hi = min(L, t0 + TPC + HALO)
        xh[lo - (t0 - HALO):hi - (t0 - HALO)] = x[b, lo:hi]
        xt = _to_fp8(np.ascontiguousarray(xh.T))              # [768, 136]
        xbc = np.ascontiguousarray(x[b, t0:t0 + TPC].T).astype(bf)
        # G-edge preds: invalid halo token columns (per chunk, 4 cols)
        pgl = np.full((128, GC * 4), 0 if right else 1, np.uint8)
        pgr = np.full((128, GC * 4), 1 if right else 0, np.uint8)
        # muti merge preds: per window, 1s on the `half` trailing-edge slots
        pmf = np.zeros((128, NW * HC * 4), np.uint8)
        pmb = np.zeros((128, NW * HC * 4), np.uint8)
        for wi, w in enumerate(WINDOWS):
            half = w // 2
            for k in range(HC):
                base = (wi * HC + k) * 4
                if right:  # fwd trailing at row end: slots 4-half..3
                    pmf[:, base + 4 - half:base + 4] = 1
                else:      # bwd trailing at row start: slots 0..half-1
                    pmb[:, base:base + half] = 1
        in_maps.append({
            "xt": xt, "xb": xbc,
            "wih": wih, "whh": whh, "whh2": whh2, "bias": biasv,
            "linw": linw, "linb": linb,
            "id8": id8, "idb": idb, "ones": ones, "onesr": onesr,
            "pgl": pgl, "pgr": pgr, "pmf": pmf, "pmb": pmb,
        })
    return in_maps


def kernel(**inputs) -> np.ndarray:
    if "nc" not in _CACHE:
        _CACHE["nc"] = _build()
    nc = _CACHE["nc"]
    in_maps = _host_prep(inputs)
    res = run_bass_kernel_spmd(nc, in_maps, core_ids=list(range(NCORES)))
    out = np.empty((B, L, NL), np.float32)
    for core in range(NCORES):
        b = core // 2
        t0 = (core % 2) * TPC
        out[b, t0:t0 + TPC] = res.results[core]["out"].T
    return out
